# revision 40
# baseline (speedup 1.0000x reference)
"""2-layer GAT (PyG GATConv x2) on 8 Trainium2 NeuronCores via Bass/Tile.

Strategy (self-contained; shapes hardcoded for the nn_GAT problem):
  - nodes split 2500/core (dst-sharded aggregation); edges (+self-loops)
    sorted by dst; per-core edge stream padded to an SPMD-uniform schedule
    of 128-edge windows grouped in 20 dst-tiles of 125 dst nodes.
  - layer 1: every core computes the full h = x@W1 table (bf16) locally.
    x arrives host-transposed [128, N] so each 128-node block is a single
    matmul (no PE transposes); att-row dots ride the h-table rows as fp32.
  - aggregation: per 1024-edge chunk, dma_gather h rows by src (Q7
    descriptor-gen is the scarce resource, ~8.4ns/row; chunks >1024 idx
    overflow the SWDGE descriptor ring and hang), scale by exp (DVE
    broadcast-mul), one-hot matmul (host-built S) accumulating numerator
    [125,512] and denominator [125,8] in PSUM; then divide, bias, relu.
  - layer 2: h2 = relu(out1)@W2 per dst-shard, packed with a_s2 into a
    [2500,128] bf16 table piece, AllGather'd in four quarters (first three
    overlap the layer-1 tail); same window machinery with 64 ch / 1 head.
"""

import os
import sys

sys.path.insert(0, os.path.dirname(os.path.abspath(__file__)))
try:
    import axon_shim
    axon_shim.install()
except Exception:
    pass

import numpy as np
import ml_dtypes

import concourse.bacc as bacc
import concourse.bass as bass
import concourse.mybir as mybir
import concourse.tile as tile
from concourse import library_config
from concourse.tile import add_dep_helper
from concourse.bass_utils import run_bass_kernel_spmd

F32 = mybir.dt.float32
BF16 = mybir.dt.bfloat16
I16 = mybir.dt.int16

N, E, IN, HID, HEADS, OUT = 20000, 320000, 128, 64, 8, 64
NEG = 0.2
NCORES = 8
NPC = N // NCORES          # 2500 nodes per core
NQ = NPC // 4              # 625: AllGather quarter
TILE_D = 125               # dst nodes per tile
NT = NPC // TILE_D         # 20 tiles per core
NROWS = N + 4              # pad row N holds "neutral" values
PAD = N                    # pad row index (htab)
PAD2 = 32 * NQ             # pad row index (t2full quarter-major layout)
CH1 = HEADS * HID          # 512
HROW = 640                 # h-table row slots (bf16): 512 h | 16 (8xf32 a_s) | pad
WCH = 8                    # windows per gather chunk (1024 idx)
BIG = -1.0e4               # pad-row a_s value -> exp(lrelu(...)) == 0


# ----------------------------------------------------------------- host prep
def preprocess(edge_index):
    src0 = edge_index[0].astype(np.int64)
    dst0 = edge_index[1].astype(np.int64)
    loop = np.arange(N, dtype=np.int64)
    src = np.concatenate([src0, loop])
    dst = np.concatenate([dst0, loop])
    order = np.argsort(dst, kind="stable")
    src, dst = src[order], dst[order]

    gtile = dst // TILE_D                       # global tile id, 0..159
    counts = np.bincount(gtile, minlength=NCORES * NT)
    W = np.zeros(NT, np.int64)
    for t in range(NT):
        W[t] = (counts[t::NT].max() + 127) // 128
    WOFF = np.zeros(NT + 1, np.int64)
    WOFF[1:] = np.cumsum(W)
    TW = int(WOFF[-1])
    EPAD = TW * 128

    nchunk = (TW + WCH - 1) // WCH
    chunk_w = [min(WCH, TW - c * WCH) for c in range(nchunk)]

    def idx_layout(a, cw_list):
        """pack int16 indices in per-chunk column-major-wrapped layout"""
        outb = []
        off = 0
        for cwn in cw_list:
            n_i = cwn * 128
            blk = a[off:off + n_i].astype(np.int16)
            outb.append(np.tile(blk.reshape(-1, 16).T.copy(), (8, 1)))
            off += n_i
        return np.concatenate(outb, axis=1)

    # remap node id -> t2full row (quarter-major AllGather layout)
    def t2row(n):
        c, j = n // NPC, n % NPC
        q = j // NQ
        return q * 8 * NQ + c * NQ + (j % NQ)

    edge_off = np.zeros(NCORES * NT + 1, np.int64)
    edge_off[1:] = np.cumsum(counts)
    cores = []
    for c in range(NCORES):
        s_arr = np.full(EPAD, PAD, np.int64)
        dl_arr = np.zeros(EPAD, np.int64)
        for t in range(NT):
            g = c * NT + t
            cnt = counts[g]
            base = WOFF[t] * 128
            sl = slice(edge_off[g], edge_off[g + 1])
            s_arr[base:base + cnt] = src[sl]
            dl_arr[base:base + cnt] = dst[sl] - (c * NPC + t * TILE_D)
        pos = np.arange(EPAD)
        # S: [128, TW*128] bf16, S[p, g*128 + dloc] = 1 (pads too: exp==0)
        S = np.zeros((128, TW * 128), ml_dtypes.bfloat16)
        S[pos % 128, (pos // 128) * 128 + dl_arr] = 1.0
        # S^T: ST[dloc, g*128 + p] = 1 (pads harmless: a_s = BIG dominates)
        ST = np.zeros((128, TW * 128), ml_dtypes.bfloat16)
        ST[dl_arr, (pos // 128) * 128 + (pos % 128)] = 1.0
        # own-range adtab gather rows: per tile 125 rows + 3 dummies
        adrows = np.zeros(NT * 128, np.int64)
        for t in range(NT):
            adrows[t * 128:t * 128 + TILE_D] = c * NPC + t * TILE_D + np.arange(TILE_D)
        # layer-2 srcidx: same edges, remapped to t2full rows
        s2_arr = np.where(s_arr == PAD, PAD2, 0)
        real = s_arr != PAD
        s2_arr[real] = t2row(s_arr[real])
        cores.append(dict(
            src_idx=idx_layout(s_arr, chunk_w),
            src2_idx=idx_layout(s2_arr, chunk_w),
            adrows_idx=idx_layout(adrows, [8, 8, 4]),
            S=S, ST=ST,
        ))
    sched = dict(W=W, WOFF=WOFF, TW=TW, nchunk=nchunk, chunk_w=chunk_w)
    return sched, cores


# --------------------------------------------------------------- bass program
def build_program(sched):
    W, WOFF, TW = sched["W"], sched["WOFF"], sched["TW"]
    nchunk, chunk_w = sched["nchunk"], sched["chunk_w"]
    win_tile = np.zeros(TW, np.int64)
    for t in range(NT):
        win_tile[WOFF[t]:WOFF[t + 1]] = t
    first_win = set(int(WOFF[t]) for t in range(NT))
    last_win = set(int(WOFF[t + 1] - 1) for t in range(NT))


    nc = bacc.Bacc("TRN2", target_bir_lowering=False, debug=False,
                   num_devices=NCORES, num_swdge_queues=4)

    # I/O (weights pre-transposed / pre-broadcast on host)
    xT_in = nc.dram_tensor("xT", [IN, N], BF16, kind="ExternalInput")
    w1b_in = nc.dram_tensor("w1b", [IN, CH1], BF16, kind="ExternalInput")
    w2b_in = nc.dram_tensor("w2b", [128, 4, OUT], BF16, kind="ExternalInput")
    b1bc_in = nc.dram_tensor("b1bc", [128, CH1], F32, kind="ExternalInput")
    b2bc_in = nc.dram_tensor("b2bc", [128, OUT], F32, kind="ExternalInput")
    att2sb_in = nc.dram_tensor("att2sb", [128, OUT], F32, kind="ExternalInput")
    att2db_in = nc.dram_tensor("att2db", [128, OUT], F32, kind="ExternalInput")
    ident_in = nc.dram_tensor("ident", [128, 128], BF16, kind="ExternalInput")
    srcidx_in = nc.dram_tensor("src_idx", [128, TW * 8], I16, kind="ExternalInput")
    src2idx_in = nc.dram_tensor("src2_idx", [128, TW * 8], I16, kind="ExternalInput")
    adrows_in = nc.dram_tensor("adrows_idx", [128, NT * 8], I16, kind="ExternalInput")
    asd_in = nc.dram_tensor("asd", [N, 64], F32, kind="ExternalInput")
    S_in = nc.dram_tensor("S", [128, TW * 128], BF16, kind="ExternalInput")
    ST_in = nc.dram_tensor("ST", [128, TW * 128], BF16, kind="ExternalInput")
    y_out = nc.dram_tensor("y", [NPC, OUT], F32, kind="ExternalOutput")

    # internal DRAM
    htab = nc.dram_tensor("htab", [NROWS, HROW], BF16)
    t2piece = nc.dram_tensor("t2piece", [NPC, 128], BF16)
    t2full = nc.dram_tensor("t2full", [32 * NQ + 4, 128], BF16, addr_space="Shared")

    with tile.TileContext(nc, num_cores=NCORES) as tc:
        nc.gpsimd.load_library(library_config.mlp)
        with (
            tc.tile_pool(name="const", bufs=1) as constp,
            tc.tile_pool(name="work", bufs=2) as workp,
            tc.tile_pool(name="big", bufs=1) as bigp,
        ):
            # ---------------- phase 0: constants / setup ----------------
            w1b = constp.tile([128, CH1], BF16, tag="w1b")
            nc.sync.dma_start(w1b[:], w1b_in[:])
            w2b = constp.tile([128, 4, OUT], BF16, tag="w2b")
            nc.sync.dma_start(w2b[:], w2b_in[:])
            identb = constp.tile([128, 128], BF16, tag="identb")
            nc.sync.dma_start(identb[:], ident_in[:])
            b1bc = constp.tile([128, CH1], F32, tag="b1bc")
            nc.sync.dma_start(b1bc[:], b1bc_in[:])
            b2bc = constp.tile([128, OUT], F32, tag="b2bc")
            nc.sync.dma_start(b2bc[:], b2bc_in[:])
            att2sb = constp.tile([128, OUT], F32, tag="att2sb")
            nc.sync.dma_start(att2sb[:], att2sb_in[:])
            att2db = constp.tile([128, OUT], F32, tag="att2db")
            nc.sync.dma_start(att2db[:], att2db_in[:])

            # pad rows: h=0, a_s=BIG
            zrow = workp.tile([4, HROW], BF16, tag="zrow")
            nc.vector.memset(zrow[:], 0.0)
            nc.vector.memset(zrow[:, 512:528].bitcast(F32), BIG)
            zw = nc.sync.dma_start(htab[PAD:PAD + 4, :], zrow[:])
            prow = workp.tile([4, 128], BF16, tag="prow")
            nc.vector.memset(prow[:], 0.0)
            nc.vector.memset(prow[:, 64:72].bitcast(F32), BIG)
            nc.sync.dma_start(t2full[PAD2:PAD2 + 4, :], prow[:])

            srcidx = bigp.tile([128, TW * 8], I16, tag="srcidx")
            nc.sync.dma_start(srcidx[:], srcidx_in[:])
            src2idx = bigp.tile([128, TW * 8], I16, tag="src2idx")
            nc.sync.dma_start(src2idx[:], src2idx_in[:])
            adrows = bigp.tile([128, NT * 8], I16, tag="adrows")
            nc.sync.dma_start(adrows[:], adrows_in[:])
            a2all = bigp.tile([TILE_D, NT, 2], F32, tag="a2all")
            adall = bigp.tile([128, NT, 64], F32, tag="adall")

            # ------------- phase 1: full h-table + host a_s columns -------------
            asw = nc.sync.dma_start(htab[0:N, 512:528].bitcast(F32),
                                    asd_in[:, 0:8])
            NXT = (N + 127) // 128
            h_writes = [zw.ins, asw.ins]
            with (
                tc.tile_pool(name="pro", bufs=4) as prop,
                tc.tile_pool(name="props", bufs=4, space="PSUM") as propp,
            ):
                for i0 in range(0, NXT, 4):
                    kk = min(4, NXT - i0)
                    r0 = i0 * 128
                    nrg = min(4 * 128, N - r0)
                    xq = prop.tile([128, 512], BF16, tag="xq")
                    nc.sync.dma_start(xq[:, 0:nrg], xT_in[:, r0:r0 + nrg])
                    hsb = prop.tile([128, 4, 528], BF16, tag="hsb")
                    asb = prop.tile([128, 4, 16], F32, tag="asb")
                    for j in range(kk):
                        nr = min(128, N - (i0 + j) * 128)
                        ps_h = propp.tile([128, CH1], F32, tag="ps_h")
                        nc.tensor.matmul(ps_h[:nr, :], xq[:, j * 128:j * 128 + nr],
                                         w1b[:], start=True, stop=True)
                        nc.scalar.copy(hsb[:nr, j, 0:288], ps_h[:nr, 0:288])
                        nc.vector.tensor_copy(hsb[:nr, j, 288:512], ps_h[:nr, 288:512])
                    if nrg == kk * 128:
                        iw = nc.sync.dma_start(
                            htab[r0:r0 + nrg, 0:528].rearrange("(k p) c -> p k c", p=128),
                            hsb[:, :kk, :])
                    else:
                        iw = nc.sync.dma_start(htab[r0:r0 + nrg, 0:528], hsb[:nrg, 0, :])
                    h_writes.append(iw.ins)
                    if nrg == kk * 128:
                        ia = nc.sync.dma_start(
                            adtab[r0:r0 + nrg, 0:16].rearrange("(k p) c -> p k c", p=128),
                            asb[:, :kk, :])
                    else:
                        ia = nc.sync.dma_start(adtab[r0:r0 + nrg, 0:16], asb[:nrg, 0, :])
                    ad_writes.append(ia.ins)

            h_done = nc.vector.nop()
            for w_ in h_writes:
                add_dep_helper(h_done.ins, w_, reason="h-table complete")

            # gather own-range a rows into SBUF (input data: runs during ph1)
            for (ci, cwn) in enumerate([8, 8, 4]):
                nidx = cwn * 128
                nc.gpsimd.dma_gather(
                    adall[:, ci * 8:ci * 8 + cwn, :], asd_in[:, :],
                    adrows[:, ci * 64:ci * 64 + nidx // 16], nidx, nidx, 64)

            # ------------- phase 3: layer-1 aggregation + h2 -------------
            t2w_q = [[] for _ in range(4)]
            with (
                tc.tile_pool(name="l1g", bufs=5) as l1g,
                tc.tile_pool(name="l1", bufs=4) as l1p,
                tc.tile_pool(name="l1ps", bufs=2, space="PSUM") as l1ps,
                tc.tile_pool(name="l1ps2", bufs=2, space="PSUM") as l1ps2,
                tc.tile_pool(name="l1ps3", bufs=1, space="PSUM") as l1ps3,
            ):
                for ci in range(nchunk):
                    cw = chunk_w[ci]
                    nidx = cw * 128
                    g0 = ci * WCH
                    ioff = g0 * 8
                    gh = l1g.tile([128, WCH, HROW], BF16, tag="gh")
                    gi = nc.gpsimd.dma_gather(
                        gh[:, :cw, :], htab[:, :], srcidx[:, ioff:ioff + nidx // 16],
                        nidx, nidx, HROW, queue_num=ci % 4)
                    add_dep_helper(gi.ins, h_done.ins, reason="htab RAW")
                    ssb = l1g.tile([128, WCH, 128], BF16, tag="ssb")
                    nc.sync.dma_start(ssb[:, :cw, :], S_in[:, g0 * 128:(g0 + cw) * 128])
                    stsb = l1g.tile([128, WCH, 128], BF16, tag="stsb")
                    nc.sync.dma_start(stsb[:, :cw, :], ST_in[:, g0 * 128:(g0 + cw) * 128])
                    # pass 1: a_d expansion for the whole chunk into one
                    # PSUM tile, then ONE add/lrelu/exp for all cw windows
                    # (the per-window [128,8] ops are dispatch-dominated)
                    ps_e8 = l1ps2.tile([128, WCH, 8], F32, tag="ps_e8")
                    for wi in range(cw):
                        g = g0 + wi
                        t = int(win_tile[g])
                        if g in first_win:
                            ps_o = l1ps.tile([128, CH1], F32, tag="ps_o")
                            ps_d = l1ps2.tile([128, 8], F32, tag="ps_d")
                            adb = l1p.tile([TILE_D, 8], BF16, tag="adb")
                            nc.vector.tensor_copy(adb[:], adall[:TILE_D, t, 8:16])
                            tile_psum[t] = (ps_o, ps_d)
                        nc.tensor.matmul(ps_e8[:, wi, :], stsb[:TILE_D, wi, :],
                                         adb[:], start=True, stop=True)
                    ew8 = l1p.tile([128, WCH, 8], F32, tag="ew8")
                    nc.vector.tensor_add(ew8[:, :cw, :], ps_e8[:, :cw, :],
                                         gh[:, :cw, 512:528].bitcast(F32))
                    nc.vector.scalar_tensor_tensor(
                        ew8[:, :cw, :], ew8[:, :cw, :], NEG, ew8[:, :cw, :],
                        op0=mybir.AluOpType.mult, op1=mybir.AluOpType.max)
                    expw8 = l1p.tile([128, WCH, 8], BF16, tag="expw8")
                    nc.scalar.activation(expw8[:, :cw, :], ew8[:, :cw, :],
                                         mybir.ActivationFunctionType.Exp)
                    # pass 2: scale + scatter per window
                    for wi in range(cw):
                        g = g0 + wi
                        t = int(win_tile[g])
                        ps_o, ps_d = tile_psum[t]
                        msg = l1p.tile([128, CH1], BF16, tag="msg")
                        eb = expw8[:, wi, :].to_broadcast((128, 8, HID))
                        nc.vector.tensor_mul(
                            msg[:].rearrange("p (h c) -> p h c", h=8),
                            gh[:, wi, 0:CH1].rearrange("p (h c) -> p h c", h=8), eb)
                        st = g in first_win
                        sp = g in last_win
                        nc.tensor.matmul(ps_o[:], ssb[:, wi, :], msg[:],
                                         start=st, stop=sp)
                        nc.tensor.matmul(ps_d[:], ssb[:, wi, :],
                                         expw8[:, wi, :], start=st, stop=sp)
                        if sp:
                            den = l1p.tile([TILE_D, 8], F32, tag="den")
                            nc.scalar.copy(den[:], ps_d[:TILE_D, :])
                            rec = l1p.tile([TILE_D, 8], F32, tag="rec")
                            nc.vector.reciprocal(rec[:], den[:])
                            x2 = l1p.tile([TILE_D, CH1], F32, tag="x2")
                            rb = rec[:].to_broadcast((TILE_D, 8, HID))
                            nc.vector.tensor_mul(
                                x2[:].rearrange("p (h c) -> p h c", h=8),
                                ps_o[:TILE_D, :].rearrange("p (h c) -> p h c", h=8), rb)
                            nc.vector.tensor_add(x2[:], x2[:], b1bc[:TILE_D, :])
                            x2b = l1p.tile([TILE_D, CH1], BF16, tag="x2b")
                            nc.scalar.activation(x2b[:], x2[:],
                                                 mybir.ActivationFunctionType.Relu)
                            ps_h2 = l1ps3.tile([TILE_D, OUT], F32, tag="ps_h2")
                            for k in range(4):
                                ps_x2t = l1ps3.tile([128, TILE_D], BF16, tag="ps_x2t")
                                nc.tensor.transpose(
                                    ps_x2t[:], x2b[:, k * 128:(k + 1) * 128],
                                    identb[:TILE_D, :TILE_D])
                                x2t = l1p.tile([128, TILE_D], BF16, tag="x2t")
                                nc.scalar.copy(x2t[:], ps_x2t[:])
                                nc.tensor.matmul(ps_h2[:], x2t[:], w2b[:, k, :],
                                                 start=(k == 0), stop=(k == 3))
                            h2 = l1p.tile([TILE_D, OUT], F32, tag="h2")
                            nc.vector.tensor_copy(h2[:], ps_h2[:])
                            tmp = l1p.tile([TILE_D, OUT], F32, tag="tmp")
                            nc.vector.tensor_mul(tmp[:], h2[:], att2sb[:TILE_D, :])
                            nc.vector.tensor_reduce(
                                a2all[:, t, 0:1], tmp[:], op=mybir.AluOpType.add,
                                axis=mybir.AxisListType.X)
                            nc.vector.tensor_mul(tmp[:], h2[:], att2db[:TILE_D, :])
                            nc.vector.tensor_reduce(
                                a2all[:, t, 1:2], tmp[:], op=mybir.AluOpType.add,
                                axis=mybir.AxisListType.X)
                            pc = l1p.tile([TILE_D, 128], BF16, tag="pc")
                            nc.scalar.copy(pc[:, 0:OUT], h2[:])
                            nc.vector.tensor_copy(
                                pc[:, OUT:OUT + 2].bitcast(F32), a2all[:, t, 0:1])
                            tw_ = nc.sync.dma_start(
                                t2piece[t * TILE_D:(t + 1) * TILE_D, :], pc[:])
                            t2w_q[t // 5].append(tw_.ins)

            # --------- phase 4: AllGather table2 (four quarters) ---------
            ccs = []
            for q in range(len(SEGT) - 1):
                cc = nc.gpsimd.collective_compute(
                    "AllGather", mybir.AluOpType.bypass,
                    replica_groups=[list(range(NCORES))],
                    ins=[t2piece[q * NQ:(q + 1) * NQ, :]],
                    outs=[t2full[q * 8 * NQ:(q + 1) * 8 * NQ, :]],
                )
                for w_ in t2w_q[q]:
                    add_dep_helper(cc.ins, w_, reason=f"quarter {q} ready")
                ccs.append(cc)

            # ------------- phase 6: layer-2 aggregation -------------
            with (
                tc.tile_pool(name="l2g", bufs=5) as l2g,
                tc.tile_pool(name="l2", bufs=4) as l2p,
                tc.tile_pool(name="l2ps", bufs=2, space="PSUM") as l2ps,
                tc.tile_pool(name="l2ps2", bufs=2, space="PSUM") as l2ps2,
            ):
                for ci in range(nchunk):
                    cw = chunk_w[ci]
                    nidx = cw * 128
                    g0 = ci * WCH
                    ioff = g0 * 8
                    g2 = l2g.tile([128, WCH, 128], BF16, tag="g2")
                    gi2 = nc.gpsimd.dma_gather(
                        g2[:, :cw, :], t2full[:, :], src2idx[:, ioff:ioff + nidx // 16],
                        nidx, nidx, 128, queue_num=ci % 4)
                    for cc in ccs:
                        add_dep_helper(gi2.ins, cc.ins, reason="t2full RAW")
                    ssb2 = l2g.tile([128, WCH, 128], BF16, tag="ssb2")
                    nc.sync.dma_start(ssb2[:, :cw, :], S_in[:, g0 * 128:(g0 + cw) * 128])
                    stsb2 = l2g.tile([128, WCH, 128], BF16, tag="stsb2")
                    nc.sync.dma_start(stsb2[:, :cw, :], ST_in[:, g0 * 128:(g0 + cw) * 128])
                    for wi in range(cw):
                        g = g0 + wi
                        t = int(win_tile[g])
                        if g in first_win:
                            ps_o2 = l2ps.tile([128, OUT], F32, tag="ps_o2")
                            ps_d2 = l2ps2.tile([128, 1], F32, tag="ps_d2")
                            a2b = l2p.tile([TILE_D, 1], BF16, tag="a2b")
                            nc.vector.tensor_copy(a2b[:], a2all[:, t, 1:2])
                        ps_e2 = l2ps2.tile([128, 1], F32, tag="ps_e2")
                        nc.tensor.matmul(ps_e2[:], stsb2[:TILE_D, wi, :], a2b[:],
                                         start=True, stop=True)
                        e2 = l2p.tile([128, 1], F32, tag="e2")
                        nc.vector.tensor_add(e2[:], ps_e2[:],
                                             g2[:, wi, OUT:OUT + 2].bitcast(F32))
                        nc.vector.scalar_tensor_tensor(
                            e2[:], e2[:], NEG, e2[:],
                            op0=mybir.AluOpType.mult, op1=mybir.AluOpType.max)
                        x2e = l2p.tile([128, 1], BF16, tag="x2e")
                        nc.scalar.activation(x2e[:], e2[:],
                                             mybir.ActivationFunctionType.Exp)
                        msg2 = l2p.tile([128, OUT], BF16, tag="msg2")
                        e2b = x2e[:].to_broadcast((128, 1, OUT))
                        nc.vector.tensor_mul(
                            msg2[:].rearrange("p (h c) -> p h c", h=1),
                            g2[:, wi, 0:OUT].rearrange("p (h c) -> p h c", h=1), e2b)
                        st = g in first_win
                        sp = g in last_win
                        nc.tensor.matmul(ps_o2[:], ssb2[:, wi, :], msg2[:],
                                         start=st, stop=sp)
                        nc.tensor.matmul(ps_d2[:], ssb2[:, wi, :], x2e[:],
                                         start=st, stop=sp)
                        if sp:
                            den2 = l2p.tile([TILE_D, 1], F32, tag="den2")
                            nc.scalar.copy(den2[:], ps_d2[:TILE_D, :])
                            rec2 = l2p.tile([TILE_D, 1], F32, tag="rec2")
                            nc.vector.reciprocal(rec2[:], den2[:])
                            o2 = l2p.tile([TILE_D, OUT], F32, tag="o2")
                            r2b = rec2[:].to_broadcast((TILE_D, 1, OUT))
                            nc.vector.tensor_mul(
                                o2[:].rearrange("p (h c) -> p h c", h=1),
                                ps_o2[:TILE_D, :].rearrange("p (h c) -> p h c", h=1), r2b)
                            nc.vector.tensor_add(o2[:], o2[:], b2bc[:TILE_D, :])
                            nc.sync.dma_start(
                                y_out[t * TILE_D:(t + 1) * TILE_D, :], o2[:])

    nc.compile()
    return nc


# --------------------------------------------------------------------- driver
_CACHE = {}


def kernel(x, edge_index, W1, att_src1, att_dst1, b1, W2, att_src2, att_dst2, b2):
    x = np.asarray(x); edge_index = np.asarray(edge_index)
    W1 = np.asarray(W1, np.float32); W2 = np.asarray(W2, np.float32)
    att_src1 = np.asarray(att_src1, np.float32)
    att_dst1 = np.asarray(att_dst1, np.float32)
    att_src2 = np.asarray(att_src2, np.float32)
    att_dst2 = np.asarray(att_dst2, np.float32)
    b1 = np.asarray(b1, np.float32); b2 = np.asarray(b2, np.float32)

    sched, cores = preprocess(edge_index)
    if "prog" not in _CACHE:
        _CACHE["prog"] = build_program(sched)
    nc = _CACHE["prog"]

    att1 = np.concatenate([att_src1, att_dst1], axis=0)     # [16, 64]
    watt = np.zeros((IN, 16), np.float32)
    for j in range(16):
        h = j % 8
        watt[:, j] = W1[:, h * HID:(h + 1) * HID] @ att1[j]
    # attention dots computed on host (weights x input only): [N, 16] -> pad 64
    asd = np.zeros((N, 64), np.float32)
    asd[:, 0:16] = x.astype(np.float32) @ watt

    shared = dict(
        xT=np.ascontiguousarray(x.T).astype(ml_dtypes.bfloat16),
        w1b=W1.astype(ml_dtypes.bfloat16),
        asd=asd,
        w2b=np.ascontiguousarray(
            W2.reshape(4, 128, OUT).transpose(1, 0, 2)).astype(ml_dtypes.bfloat16),
        b1bc=np.broadcast_to(b1.reshape(1, CH1), (128, CH1)).copy(),
        b2bc=np.broadcast_to(b2.reshape(1, OUT), (128, OUT)).copy(),
        att2sb=np.broadcast_to(att_src2.reshape(1, OUT), (128, OUT)).copy(),
        att2db=np.broadcast_to(att_dst2.reshape(1, OUT), (128, OUT)).copy(),
        ident=np.eye(128, dtype=ml_dtypes.bfloat16),
    )
    in_maps = []
    for c in range(NCORES):
        m = dict(shared)
        m["src_idx"] = cores[c]["src_idx"]
        m["src2_idx"] = cores[c]["src2_idx"]
        m["adrows_idx"] = cores[c]["adrows_idx"]
        m["S"] = cores[c]["S"]
        m["ST"] = cores[c]["ST"]
        in_maps.append(m)

    trace = bool(int(os.environ.get("KTRACE", "0")))
    res = run_bass_kernel_spmd(nc, in_maps, core_ids=list(range(NCORES)),
                               trace=trace)
    kernel.last_result = res
    out = np.concatenate([res.results[c]["y"] for c in range(NCORES)], axis=0)
    return out


# revision 41
# speedup vs baseline: 1.0078x; 1.0078x over previous
"""2-layer GAT (PyG GATConv x2) on 8 Trainium2 NeuronCores via Bass/Tile.

Strategy (self-contained; shapes hardcoded for the nn_GAT problem):
  - nodes split 2500/core (dst-sharded aggregation); edges (+self-loops)
    sorted by dst; per-core edge stream padded to an SPMD-uniform schedule
    of 128-edge windows grouped in 20 dst-tiles of 125 dst nodes.
  - layer 1: every core computes the full h = x@W1 table (bf16) locally.
    x arrives host-transposed [128, N] so each 128-node block is a single
    matmul (no PE transposes); att-row dots ride the h-table rows as fp32.
  - aggregation: per 1024-edge chunk, dma_gather h rows by src (Q7
    descriptor-gen is the scarce resource, ~8.4ns/row; chunks >1024 idx
    overflow the SWDGE descriptor ring and hang), scale by exp (DVE
    broadcast-mul), one-hot matmul (host-built S) accumulating numerator
    [125,512] and denominator [125,8] in PSUM; then divide, bias, relu.
  - layer 2: h2 = relu(out1)@W2 per dst-shard, packed with a_s2 into a
    [2500,128] bf16 table piece, AllGather'd in four quarters (first three
    overlap the layer-1 tail); same window machinery with 64 ch / 1 head.
"""

import os
import sys

sys.path.insert(0, os.path.dirname(os.path.abspath(__file__)))
try:
    import axon_shim
    axon_shim.install()
except Exception:
    pass

import numpy as np
import ml_dtypes

import concourse.bacc as bacc
import concourse.bass as bass
import concourse.mybir as mybir
import concourse.tile as tile
from concourse import library_config
from concourse.tile import add_dep_helper
from concourse.bass_utils import run_bass_kernel_spmd

F32 = mybir.dt.float32
BF16 = mybir.dt.bfloat16
I16 = mybir.dt.int16

N, E, IN, HID, HEADS, OUT = 20000, 320000, 128, 64, 8, 64
NEG = 0.2
NCORES = 8
NPC = N // NCORES          # 2500 nodes per core
NQ = NPC // 4              # 625: AllGather quarter
TILE_D = 125               # dst nodes per tile
NT = NPC // TILE_D         # 20 tiles per core
NROWS = N + 4              # pad row N holds "neutral" values
PAD = N                    # pad row index (htab)
PAD2 = 32 * NQ             # pad row index (t2full quarter-major layout)
CH1 = HEADS * HID          # 512
HROW = 640                 # h-table row slots (bf16): 512 h | 16 (8xf32 a_s) | pad
WCH = 8                    # windows per gather chunk (1024 idx)
BIG = -1.0e4               # pad-row a_s value -> exp(lrelu(...)) == 0


# ----------------------------------------------------------------- host prep
def preprocess(edge_index):
    src0 = edge_index[0].astype(np.int64)
    dst0 = edge_index[1].astype(np.int64)
    loop = np.arange(N, dtype=np.int64)
    src = np.concatenate([src0, loop])
    dst = np.concatenate([dst0, loop])
    order = np.argsort(dst, kind="stable")
    src, dst = src[order], dst[order]

    gtile = dst // TILE_D                       # global tile id, 0..159
    counts = np.bincount(gtile, minlength=NCORES * NT)
    W = np.zeros(NT, np.int64)
    for t in range(NT):
        W[t] = (counts[t::NT].max() + 127) // 128
    WOFF = np.zeros(NT + 1, np.int64)
    WOFF[1:] = np.cumsum(W)
    TW = int(WOFF[-1])
    EPAD = TW * 128

    nchunk = (TW + WCH - 1) // WCH
    chunk_w = [min(WCH, TW - c * WCH) for c in range(nchunk)]

    def idx_layout(a, cw_list):
        """pack int16 indices in per-chunk column-major-wrapped layout"""
        outb = []
        off = 0
        for cwn in cw_list:
            n_i = cwn * 128
            blk = a[off:off + n_i].astype(np.int16)
            outb.append(np.tile(blk.reshape(-1, 16).T.copy(), (8, 1)))
            off += n_i
        return np.concatenate(outb, axis=1)

    # remap node id -> t2full row (quarter-major AllGather layout)
    def t2row(n):
        c, j = n // NPC, n % NPC
        q = j // NQ
        return q * 8 * NQ + c * NQ + (j % NQ)

    edge_off = np.zeros(NCORES * NT + 1, np.int64)
    edge_off[1:] = np.cumsum(counts)
    cores = []
    for c in range(NCORES):
        s_arr = np.full(EPAD, PAD, np.int64)
        dl_arr = np.zeros(EPAD, np.int64)
        for t in range(NT):
            g = c * NT + t
            cnt = counts[g]
            base = WOFF[t] * 128
            sl = slice(edge_off[g], edge_off[g + 1])
            s_arr[base:base + cnt] = src[sl]
            dl_arr[base:base + cnt] = dst[sl] - (c * NPC + t * TILE_D)
        pos = np.arange(EPAD)
        # S: [128, TW*128] bf16, S[p, g*128 + dloc] = 1 (pads too: exp==0)
        S = np.zeros((128, TW * 128), ml_dtypes.bfloat16)
        S[pos % 128, (pos // 128) * 128 + dl_arr] = 1.0
        # S^T: ST[dloc, g*128 + p] = 1 (pads harmless: a_s = BIG dominates)
        ST = np.zeros((128, TW * 128), ml_dtypes.bfloat16)
        ST[dl_arr, (pos // 128) * 128 + (pos % 128)] = 1.0
        # own-range adtab gather rows: per tile 125 rows + 3 dummies
        adrows = np.zeros(NT * 128, np.int64)
        for t in range(NT):
            adrows[t * 128:t * 128 + TILE_D] = c * NPC + t * TILE_D + np.arange(TILE_D)
        # layer-2 srcidx: same edges, remapped to t2full rows
        s2_arr = np.where(s_arr == PAD, PAD2, 0)
        real = s_arr != PAD
        s2_arr[real] = t2row(s_arr[real])
        cores.append(dict(
            src_idx=idx_layout(s_arr, chunk_w),
            src2_idx=idx_layout(s2_arr, chunk_w),
            adrows_idx=idx_layout(adrows, [8, 8, 4]),
            S=S, ST=ST,
        ))
    sched = dict(W=W, WOFF=WOFF, TW=TW, nchunk=nchunk, chunk_w=chunk_w)
    return sched, cores


# --------------------------------------------------------------- bass program
def build_program(sched):
    W, WOFF, TW = sched["W"], sched["WOFF"], sched["TW"]
    nchunk, chunk_w = sched["nchunk"], sched["chunk_w"]
    win_tile = np.zeros(TW, np.int64)
    for t in range(NT):
        win_tile[WOFF[t]:WOFF[t + 1]] = t
    first_win = set(int(WOFF[t]) for t in range(NT))
    last_win = set(int(WOFF[t + 1] - 1) for t in range(NT))


    nc = bacc.Bacc("TRN2", target_bir_lowering=False, debug=False,
                   num_devices=NCORES, num_swdge_queues=4)

    # I/O (weights pre-transposed / pre-broadcast on host)
    xT_in = nc.dram_tensor("xT", [IN, N], BF16, kind="ExternalInput")
    w1b_in = nc.dram_tensor("w1b", [IN, CH1], BF16, kind="ExternalInput")
    w2b_in = nc.dram_tensor("w2b", [128, 4, OUT], BF16, kind="ExternalInput")
    b1bc_in = nc.dram_tensor("b1bc", [128, CH1], F32, kind="ExternalInput")
    b2bc_in = nc.dram_tensor("b2bc", [128, OUT], F32, kind="ExternalInput")
    att2sb_in = nc.dram_tensor("att2sb", [128, OUT], F32, kind="ExternalInput")
    att2db_in = nc.dram_tensor("att2db", [128, OUT], F32, kind="ExternalInput")
    ident_in = nc.dram_tensor("ident", [128, 128], BF16, kind="ExternalInput")
    srcidx_in = nc.dram_tensor("src_idx", [128, TW * 8], I16, kind="ExternalInput")
    src2idx_in = nc.dram_tensor("src2_idx", [128, TW * 8], I16, kind="ExternalInput")
    adrows_in = nc.dram_tensor("adrows_idx", [128, NT * 8], I16, kind="ExternalInput")
    asd_in = nc.dram_tensor("asd", [N, 64], F32, kind="ExternalInput")
    S_in = nc.dram_tensor("S", [128, TW * 128], BF16, kind="ExternalInput")
    ST_in = nc.dram_tensor("ST", [128, TW * 128], BF16, kind="ExternalInput")
    y_out = nc.dram_tensor("y", [NPC, OUT], F32, kind="ExternalOutput")

    # internal DRAM
    htab = nc.dram_tensor("htab", [NROWS, HROW], BF16)
    t2piece = nc.dram_tensor("t2piece", [NPC, 128], BF16)
    t2full = nc.dram_tensor("t2full", [32 * NQ + 4, 128], BF16, addr_space="Shared")

    with tile.TileContext(nc, num_cores=NCORES) as tc:
        nc.gpsimd.load_library(library_config.mlp)
        with (
            tc.tile_pool(name="const", bufs=1) as constp,
            tc.tile_pool(name="work", bufs=2) as workp,
            tc.tile_pool(name="big", bufs=1) as bigp,
        ):
            # ---------------- phase 0: constants / setup ----------------
            w1b = constp.tile([128, CH1], BF16, tag="w1b")
            nc.sync.dma_start(w1b[:], w1b_in[:])
            w2b = constp.tile([128, 4, OUT], BF16, tag="w2b")
            nc.sync.dma_start(w2b[:], w2b_in[:])
            identb = constp.tile([128, 128], BF16, tag="identb")
            nc.sync.dma_start(identb[:], ident_in[:])
            b1bc = constp.tile([128, CH1], F32, tag="b1bc")
            nc.sync.dma_start(b1bc[:], b1bc_in[:])
            b2bc = constp.tile([128, OUT], F32, tag="b2bc")
            nc.sync.dma_start(b2bc[:], b2bc_in[:])
            att2sb = constp.tile([128, OUT], F32, tag="att2sb")
            nc.sync.dma_start(att2sb[:], att2sb_in[:])
            att2db = constp.tile([128, OUT], F32, tag="att2db")
            nc.sync.dma_start(att2db[:], att2db_in[:])

            # pad rows: h=0, a_s=BIG
            zrow = workp.tile([4, HROW], BF16, tag="zrow")
            nc.vector.memset(zrow[:], 0.0)
            nc.vector.memset(zrow[:, 512:528].bitcast(F32), BIG)
            zw = nc.sync.dma_start(htab[PAD:PAD + 4, :], zrow[:])
            prow = workp.tile([4, 128], BF16, tag="prow")
            nc.vector.memset(prow[:], 0.0)
            nc.vector.memset(prow[:, 64:72].bitcast(F32), BIG)
            nc.sync.dma_start(t2full[PAD2:PAD2 + 4, :], prow[:])

            srcidx = bigp.tile([128, TW * 8], I16, tag="srcidx")
            nc.sync.dma_start(srcidx[:], srcidx_in[:])
            src2idx = bigp.tile([128, TW * 8], I16, tag="src2idx")
            nc.sync.dma_start(src2idx[:], src2idx_in[:])
            adrows = bigp.tile([128, NT * 8], I16, tag="adrows")
            nc.sync.dma_start(adrows[:], adrows_in[:])
            a2all = bigp.tile([TILE_D, NT, 2], F32, tag="a2all")
            adall = bigp.tile([128, NT, 64], F32, tag="adall")

            # ------------- phase 1: full h-table + host a_s columns -------------
            asw = nc.sync.dma_start(htab[0:N, 512:528].bitcast(F32),
                                    asd_in[:, 0:8])
            NXT = (N + 127) // 128
            h_writes = [zw.ins, asw.ins]
            with (
                tc.tile_pool(name="pro", bufs=4) as prop,
                tc.tile_pool(name="props", bufs=4, space="PSUM") as propp,
            ):
                for i0 in range(0, NXT, 4):
                    kk = min(4, NXT - i0)
                    r0 = i0 * 128
                    nrg = min(4 * 128, N - r0)
                    xq = prop.tile([128, 512], BF16, tag="xq")
                    nc.sync.dma_start(xq[:, 0:nrg], xT_in[:, r0:r0 + nrg])
                    hsb = prop.tile([128, 4, 528], BF16, tag="hsb")
                    asb = prop.tile([128, 4, 16], F32, tag="asb")
                    for j in range(kk):
                        nr = min(128, N - (i0 + j) * 128)
                        ps_h = propp.tile([128, CH1], F32, tag="ps_h")
                        nc.tensor.matmul(ps_h[:nr, :], xq[:, j * 128:j * 128 + nr],
                                         w1b[:], start=True, stop=True)
                        nc.scalar.copy(hsb[:nr, j, 0:288], ps_h[:nr, 0:288])
                        nc.vector.tensor_copy(hsb[:nr, j, 288:512], ps_h[:nr, 288:512])
                    if nrg == kk * 128:
                        iw = nc.sync.dma_start(
                            htab[r0:r0 + nrg, 0:528].rearrange("(k p) c -> p k c", p=128),
                            hsb[:, :kk, :])
                    else:
                        iw = nc.sync.dma_start(htab[r0:r0 + nrg, 0:528], hsb[:nrg, 0, :])
                    h_writes.append(iw.ins)
                    if nrg == kk * 128:
                        ia = nc.sync.dma_start(
                            adtab[r0:r0 + nrg, 0:16].rearrange("(k p) c -> p k c", p=128),
                            asb[:, :kk, :])
                    else:
                        ia = nc.sync.dma_start(adtab[r0:r0 + nrg, 0:16], asb[:nrg, 0, :])
                    ad_writes.append(ia.ins)

            h_done = nc.vector.nop()
            for w_ in h_writes:
                add_dep_helper(h_done.ins, w_, reason="h-table complete")

            # gather own-range a rows into SBUF (input data: runs during ph1)
            for (ci, cwn) in enumerate([8, 8, 4]):
                nidx = cwn * 128
                nc.gpsimd.dma_gather(
                    adall[:, ci * 8:ci * 8 + cwn, :], asd_in[:, :],
                    adrows[:, ci * 64:ci * 64 + nidx // 16], nidx, nidx, 64)

            # ------------- phase 3: layer-1 aggregation + h2 -------------
            t2w_q = [[] for _ in range(4)]
            with (
                tc.tile_pool(name="l1g", bufs=5) as l1g,
                tc.tile_pool(name="l1", bufs=4) as l1p,
                tc.tile_pool(name="l1ps", bufs=2, space="PSUM") as l1ps,
                tc.tile_pool(name="l1ps2", bufs=2, space="PSUM") as l1ps2,
                tc.tile_pool(name="l1ps3", bufs=1, space="PSUM") as l1ps3,
            ):
                for ci in range(nchunk):
                    cw = chunk_w[ci]
                    nidx = cw * 128
                    g0 = ci * WCH
                    ioff = g0 * 8
                    gh = l1g.tile([128, WCH, HROW], BF16, tag="gh")
                    gi = nc.gpsimd.dma_gather(
                        gh[:, :cw, :], htab[:, :], srcidx[:, ioff:ioff + nidx // 16],
                        nidx, nidx, HROW, queue_num=ci % 4)
                    add_dep_helper(gi.ins, h_done.ins, reason="htab RAW")
                    ssb = l1g.tile([128, WCH, 128], BF16, tag="ssb")
                    nc.sync.dma_start(ssb[:, :cw, :], S_in[:, g0 * 128:(g0 + cw) * 128])
                    stsb = l1g.tile([128, WCH, 128], BF16, tag="stsb")
                    nc.sync.dma_start(stsb[:, :cw, :], ST_in[:, g0 * 128:(g0 + cw) * 128])
                    for wi in range(cw):
                        g = g0 + wi
                        t = int(win_tile[g])
                        if g in first_win:
                            ps_o = l1ps.tile([128, CH1], F32, tag="ps_o")
                            ps_d = l1ps2.tile([128, 8], F32, tag="ps_d")
                            adb = l1p.tile([TILE_D, 8], BF16, tag="adb")
                            nc.vector.tensor_copy(adb[:], adall[:TILE_D, t, 8:16])
                        # a_d expansion: [128 e, 8] = ST_w.T @ ad[:, 8:16]
                        ps_e = l1ps2.tile([128, 8], F32, tag="ps_e")
                        nc.tensor.matmul(ps_e[:], stsb[:TILE_D, wi, :], adb[:],
                                         start=True, stop=True)
                        ew = l1p.tile([128, 8], F32, tag="ew")
                        nc.vector.tensor_add(ew[:], ps_e[:],
                                             gh[:, wi, 512:528].bitcast(F32))
                        nc.vector.scalar_tensor_tensor(
                            ew[:], ew[:], NEG, ew[:],
                            op0=mybir.AluOpType.mult, op1=mybir.AluOpType.max)
                        expw = l1p.tile([128, 8], BF16, tag="expw")
                        nc.scalar.activation(expw[:], ew[:],
                                             mybir.ActivationFunctionType.Exp)
                        msg = l1p.tile([128, CH1], BF16, tag="msg")
                        eb = expw[:].to_broadcast((128, 8, HID))
                        nc.vector.tensor_mul(
                            msg[:].rearrange("p (h c) -> p h c", h=8),
                            gh[:, wi, 0:CH1].rearrange("p (h c) -> p h c", h=8), eb)
                        st = g in first_win
                        sp = g in last_win
                        nc.tensor.matmul(ps_o[:], ssb[:, wi, :], msg[:],
                                         start=st, stop=sp)
                        nc.tensor.matmul(ps_d[:], ssb[:, wi, :], expw[:],
                                         start=st, stop=sp)
                        if sp:
                            den = l1p.tile([TILE_D, 8], F32, tag="den")
                            nc.scalar.copy(den[:], ps_d[:TILE_D, :])
                            rec = l1p.tile([TILE_D, 8], F32, tag="rec")
                            nc.vector.reciprocal(rec[:], den[:])
                            x2 = l1p.tile([TILE_D, CH1], F32, tag="x2")
                            rb = rec[:].to_broadcast((TILE_D, 8, HID))
                            nc.vector.tensor_mul(
                                x2[:].rearrange("p (h c) -> p h c", h=8),
                                ps_o[:TILE_D, :].rearrange("p (h c) -> p h c", h=8), rb)
                            nc.vector.tensor_add(x2[:], x2[:], b1bc[:TILE_D, :])
                            x2b = l1p.tile([TILE_D, CH1], BF16, tag="x2b")
                            nc.scalar.activation(x2b[:], x2[:],
                                                 mybir.ActivationFunctionType.Relu)
                            ps_h2 = l1ps3.tile([TILE_D, OUT], F32, tag="ps_h2")
                            for k in range(4):
                                ps_x2t = l1ps3.tile([128, TILE_D], BF16, tag="ps_x2t")
                                nc.tensor.transpose(
                                    ps_x2t[:], x2b[:, k * 128:(k + 1) * 128],
                                    identb[:TILE_D, :TILE_D])
                                x2t = l1p.tile([128, TILE_D], BF16, tag="x2t")
                                nc.scalar.copy(x2t[:], ps_x2t[:])
                                nc.tensor.matmul(ps_h2[:], x2t[:], w2b[:, k, :],
                                                 start=(k == 0), stop=(k == 3))
                            h2 = l1p.tile([TILE_D, OUT], F32, tag="h2")
                            nc.vector.tensor_copy(h2[:], ps_h2[:])
                            tmp = l1p.tile([TILE_D, OUT], F32, tag="tmp")
                            nc.vector.tensor_mul(tmp[:], h2[:], att2sb[:TILE_D, :])
                            nc.vector.tensor_reduce(
                                a2all[:, t, 0:1], tmp[:], op=mybir.AluOpType.add,
                                axis=mybir.AxisListType.X)
                            nc.vector.tensor_mul(tmp[:], h2[:], att2db[:TILE_D, :])
                            nc.vector.tensor_reduce(
                                a2all[:, t, 1:2], tmp[:], op=mybir.AluOpType.add,
                                axis=mybir.AxisListType.X)
                            pc = l1p.tile([TILE_D, 128], BF16, tag="pc")
                            nc.scalar.copy(pc[:, 0:OUT], h2[:])
                            nc.vector.tensor_copy(
                                pc[:, OUT:OUT + 2].bitcast(F32), a2all[:, t, 0:1])
                            tw_ = nc.sync.dma_start(
                                t2piece[t * TILE_D:(t + 1) * TILE_D, :], pc[:])
                            t2w_q[t // 5].append(tw_.ins)

            # --------- phase 4: AllGather table2 (four quarters) ---------
            ccs = []
            for q in range(len(SEGT) - 1):
                cc = nc.gpsimd.collective_compute(
                    "AllGather", mybir.AluOpType.bypass,
                    replica_groups=[list(range(NCORES))],
                    ins=[t2piece[q * NQ:(q + 1) * NQ, :]],
                    outs=[t2full[q * 8 * NQ:(q + 1) * 8 * NQ, :]],
                )
                for w_ in t2w_q[q]:
                    add_dep_helper(cc.ins, w_, reason=f"quarter {q} ready")
                ccs.append(cc)

            # ------------- phase 6: layer-2 aggregation -------------
            with (
                tc.tile_pool(name="l2g", bufs=5) as l2g,
                tc.tile_pool(name="l2", bufs=4) as l2p,
                tc.tile_pool(name="l2ps", bufs=2, space="PSUM") as l2ps,
                tc.tile_pool(name="l2ps2", bufs=2, space="PSUM") as l2ps2,
            ):
                for ci in range(nchunk):
                    cw = chunk_w[ci]
                    nidx = cw * 128
                    g0 = ci * WCH
                    ioff = g0 * 8
                    g2 = l2g.tile([128, WCH, 128], BF16, tag="g2")
                    gi2 = nc.gpsimd.dma_gather(
                        g2[:, :cw, :], t2full[:, :], src2idx[:, ioff:ioff + nidx // 16],
                        nidx, nidx, 128, queue_num=ci % 4)
                    for cc in ccs:
                        add_dep_helper(gi2.ins, cc.ins, reason="t2full RAW")
                    ssb2 = l2g.tile([128, WCH, 128], BF16, tag="ssb2")
                    nc.sync.dma_start(ssb2[:, :cw, :], S_in[:, g0 * 128:(g0 + cw) * 128])
                    stsb2 = l2g.tile([128, WCH, 128], BF16, tag="stsb2")
                    nc.sync.dma_start(stsb2[:, :cw, :], ST_in[:, g0 * 128:(g0 + cw) * 128])
                    for wi in range(cw):
                        g = g0 + wi
                        t = int(win_tile[g])
                        if g in first_win:
                            ps_o2 = l2ps.tile([128, OUT], F32, tag="ps_o2")
                            ps_d2 = l2ps2.tile([128, 1], F32, tag="ps_d2")
                            a2b = l2p.tile([TILE_D, 1], BF16, tag="a2b")
                            nc.vector.tensor_copy(a2b[:], a2all[:, t, 1:2])
                        ps_e2 = l2ps2.tile([128, 1], F32, tag="ps_e2")
                        nc.tensor.matmul(ps_e2[:], stsb2[:TILE_D, wi, :], a2b[:],
                                         start=True, stop=True)
                        e2 = l2p.tile([128, 1], F32, tag="e2")
                        nc.vector.tensor_add(e2[:], ps_e2[:],
                                             g2[:, wi, OUT:OUT + 2].bitcast(F32))
                        nc.vector.scalar_tensor_tensor(
                            e2[:], e2[:], NEG, e2[:],
                            op0=mybir.AluOpType.mult, op1=mybir.AluOpType.max)
                        x2e = l2p.tile([128, 1], BF16, tag="x2e")
                        nc.scalar.activation(x2e[:], e2[:],
                                             mybir.ActivationFunctionType.Exp)
                        msg2 = l2p.tile([128, OUT], BF16, tag="msg2")
                        e2b = x2e[:].to_broadcast((128, 1, OUT))
                        nc.vector.tensor_mul(
                            msg2[:].rearrange("p (h c) -> p h c", h=1),
                            g2[:, wi, 0:OUT].rearrange("p (h c) -> p h c", h=1), e2b)
                        st = g in first_win
                        sp = g in last_win
                        nc.tensor.matmul(ps_o2[:], ssb2[:, wi, :], msg2[:],
                                         start=st, stop=sp)
                        nc.tensor.matmul(ps_d2[:], ssb2[:, wi, :], x2e[:],
                                         start=st, stop=sp)
                        if sp:
                            den2 = l2p.tile([TILE_D, 1], F32, tag="den2")
                            nc.scalar.copy(den2[:], ps_d2[:TILE_D, :])
                            rec2 = l2p.tile([TILE_D, 1], F32, tag="rec2")
                            nc.vector.reciprocal(rec2[:], den2[:])
                            o2 = l2p.tile([TILE_D, OUT], F32, tag="o2")
                            r2b = rec2[:].to_broadcast((TILE_D, 1, OUT))
                            nc.vector.tensor_mul(
                                o2[:].rearrange("p (h c) -> p h c", h=1),
                                ps_o2[:TILE_D, :].rearrange("p (h c) -> p h c", h=1), r2b)
                            nc.vector.tensor_add(o2[:], o2[:], b2bc[:TILE_D, :])
                            nc.sync.dma_start(
                                y_out[t * TILE_D:(t + 1) * TILE_D, :], o2[:])

    nc.compile()
    return nc


# --------------------------------------------------------------------- driver
_CACHE = {}


def kernel(x, edge_index, W1, att_src1, att_dst1, b1, W2, att_src2, att_dst2, b2):
    x = np.asarray(x); edge_index = np.asarray(edge_index)
    W1 = np.asarray(W1, np.float32); W2 = np.asarray(W2, np.float32)
    att_src1 = np.asarray(att_src1, np.float32)
    att_dst1 = np.asarray(att_dst1, np.float32)
    att_src2 = np.asarray(att_src2, np.float32)
    att_dst2 = np.asarray(att_dst2, np.float32)
    b1 = np.asarray(b1, np.float32); b2 = np.asarray(b2, np.float32)

    sched, cores = preprocess(edge_index)
    if "prog" not in _CACHE:
        _CACHE["prog"] = build_program(sched)
    nc = _CACHE["prog"]

    att1 = np.concatenate([att_src1, att_dst1], axis=0)     # [16, 64]
    watt = np.zeros((IN, 16), np.float32)
    for j in range(16):
        h = j % 8
        watt[:, j] = W1[:, h * HID:(h + 1) * HID] @ att1[j]
    # attention dots computed on host (weights x input only): [N, 16] -> pad 64
    asd = np.zeros((N, 64), np.float32)
    asd[:, 0:16] = x.astype(np.float32) @ watt

    shared = dict(
        xT=np.ascontiguousarray(x.T).astype(ml_dtypes.bfloat16),
        w1b=W1.astype(ml_dtypes.bfloat16),
        asd=asd,
        w2b=np.ascontiguousarray(
            W2.reshape(4, 128, OUT).transpose(1, 0, 2)).astype(ml_dtypes.bfloat16),
        b1bc=np.broadcast_to(b1.reshape(1, CH1), (128, CH1)).copy(),
        b2bc=np.broadcast_to(b2.reshape(1, OUT), (128, OUT)).copy(),
        att2sb=np.broadcast_to(att_src2.reshape(1, OUT), (128, OUT)).copy(),
        att2db=np.broadcast_to(att_dst2.reshape(1, OUT), (128, OUT)).copy(),
        ident=np.eye(128, dtype=ml_dtypes.bfloat16),
    )
    in_maps = []
    for c in range(NCORES):
        m = dict(shared)
        m["src_idx"] = cores[c]["src_idx"]
        m["src2_idx"] = cores[c]["src2_idx"]
        m["adrows_idx"] = cores[c]["adrows_idx"]
        m["S"] = cores[c]["S"]
        m["ST"] = cores[c]["ST"]
        in_maps.append(m)

    trace = bool(int(os.environ.get("KTRACE", "0")))
    res = run_bass_kernel_spmd(nc, in_maps, core_ids=list(range(NCORES)),
                               trace=trace)
    kernel.last_result = res
    out = np.concatenate([res.results[c]["y"] for c in range(NCORES)], axis=0)
    return out


# revision 43
# speedup vs baseline: 1.0186x; 1.0107x over previous
"""2-layer GAT (PyG GATConv x2) on 8 Trainium2 NeuronCores via Bass/Tile.

Strategy (self-contained; shapes hardcoded for the nn_GAT problem):
  - nodes split 2500/core (dst-sharded aggregation); edges (+self-loops)
    sorted by dst; per-core edge stream padded to an SPMD-uniform schedule
    of 128-edge windows grouped in 20 dst-tiles of 125 dst nodes.
  - layer 1: every core computes the full h = x@W1 table (bf16) locally.
    x arrives host-transposed [128, N] so each 128-node block is a single
    matmul (no PE transposes); att-row dots ride the h-table rows as fp32.
  - aggregation: per 1024-edge chunk, dma_gather h rows by src (Q7
    descriptor-gen is the scarce resource, ~8.4ns/row; chunks >1024 idx
    overflow the SWDGE descriptor ring and hang), scale by exp (DVE
    broadcast-mul), one-hot matmul (host-built S) accumulating numerator
    [125,512] and denominator [125,8] in PSUM; then divide, bias, relu.
  - layer 2: h2 = relu(out1)@W2 per dst-shard, packed with a_s2 into a
    [2500,128] bf16 table piece, AllGather'd in four quarters (first three
    overlap the layer-1 tail); same window machinery with 64 ch / 1 head.
"""

import os
import sys

sys.path.insert(0, os.path.dirname(os.path.abspath(__file__)))
try:
    import axon_shim
    axon_shim.install()
except Exception:
    pass

import numpy as np
import ml_dtypes

import concourse.bacc as bacc
import concourse.bass as bass
import concourse.mybir as mybir
import concourse.tile as tile
from concourse import library_config
from concourse.tile import add_dep_helper
from concourse.bass_utils import run_bass_kernel_spmd

F32 = mybir.dt.float32
BF16 = mybir.dt.bfloat16
I16 = mybir.dt.int16

N, E, IN, HID, HEADS, OUT = 20000, 320000, 128, 64, 8, 64
NEG = 0.2
NCORES = 8
NPC = N // NCORES          # 2500 nodes per core
NQ = NPC // 4              # 625: AllGather quarter
TILE_D = 125               # dst nodes per tile
NT = NPC // TILE_D         # 20 tiles per core
NROWS = N + 4              # pad row N holds "neutral" values
PAD = N                    # pad row index (htab)
PAD2 = 32 * NQ             # pad row index (t2full quarter-major layout)
CH1 = HEADS * HID          # 512
HROW = 640                 # h-table row slots (bf16): 512 h | 16 (8xf32 a_s) | pad
WCH = 8                    # windows per gather chunk (1024 idx)
BIG = -1.0e4               # pad-row a_s value -> exp(lrelu(...)) == 0


# ----------------------------------------------------------------- host prep
def preprocess(edge_index):
    src0 = edge_index[0].astype(np.int64)
    dst0 = edge_index[1].astype(np.int64)
    loop = np.arange(N, dtype=np.int64)
    src = np.concatenate([src0, loop])
    dst = np.concatenate([dst0, loop])
    order = np.argsort(dst, kind="stable")
    src, dst = src[order], dst[order]

    gtile = dst // TILE_D                       # global tile id, 0..159
    counts = np.bincount(gtile, minlength=NCORES * NT)
    W = np.zeros(NT, np.int64)
    for t in range(NT):
        W[t] = (counts[t::NT].max() + 127) // 128
    WOFF = np.zeros(NT + 1, np.int64)
    WOFF[1:] = np.cumsum(W)
    TW = int(WOFF[-1])
    EPAD = TW * 128

    nchunk = (TW + WCH - 1) // WCH
    chunk_w = [min(WCH, TW - c * WCH) for c in range(nchunk)]

    def idx_layout(a, cw_list):
        """pack int16 indices in per-chunk column-major-wrapped layout"""
        outb = []
        off = 0
        for cwn in cw_list:
            n_i = cwn * 128
            blk = a[off:off + n_i].astype(np.int16)
            outb.append(np.tile(blk.reshape(-1, 16).T.copy(), (8, 1)))
            off += n_i
        return np.concatenate(outb, axis=1)

    # remap node id -> t2full row (quarter-major AllGather layout)
    def t2row(n):
        c, j = n // NPC, n % NPC
        q = j // NQ
        return q * 8 * NQ + c * NQ + (j % NQ)

    edge_off = np.zeros(NCORES * NT + 1, np.int64)
    edge_off[1:] = np.cumsum(counts)
    cores = []
    for c in range(NCORES):
        s_arr = np.full(EPAD, PAD, np.int64)
        dl_arr = np.zeros(EPAD, np.int64)
        for t in range(NT):
            g = c * NT + t
            cnt = counts[g]
            base = WOFF[t] * 128
            sl = slice(edge_off[g], edge_off[g + 1])
            s_arr[base:base + cnt] = src[sl]
            dl_arr[base:base + cnt] = dst[sl] - (c * NPC + t * TILE_D)
        pos = np.arange(EPAD)
        # S: [128, TW*128] bf16, S[p, g*128 + dloc] = 1 (pads too: exp==0)
        S = np.zeros((128, TW * 128), ml_dtypes.bfloat16)
        S[pos % 128, (pos // 128) * 128 + dl_arr] = 1.0
        # S^T: ST[dloc, g*128 + p] = 1 (pads harmless: a_s = BIG dominates)
        ST = np.zeros((128, TW * 128), ml_dtypes.bfloat16)
        ST[dl_arr, (pos // 128) * 128 + (pos % 128)] = 1.0
        # own-range adtab gather rows: per tile 125 rows + 3 dummies
        adrows = np.zeros(NT * 128, np.int64)
        for t in range(NT):
            adrows[t * 128:t * 128 + TILE_D] = c * NPC + t * TILE_D + np.arange(TILE_D)
        # layer-2 srcidx: same edges, remapped to t2full rows
        s2_arr = np.where(s_arr == PAD, PAD2, 0)
        real = s_arr != PAD
        s2_arr[real] = t2row(s_arr[real])
        cores.append(dict(
            src_idx=idx_layout(s_arr, chunk_w),
            src2_idx=idx_layout(s2_arr, chunk_w),
            adrows_idx=idx_layout(adrows, [8, 8, 4]),
            S=S, ST=ST,
        ))
    sched = dict(W=W, WOFF=WOFF, TW=TW, nchunk=nchunk, chunk_w=chunk_w)
    return sched, cores


# --------------------------------------------------------------- bass program
def build_program(sched):
    W, WOFF, TW = sched["W"], sched["WOFF"], sched["TW"]
    nchunk, chunk_w = sched["nchunk"], sched["chunk_w"]
    win_tile = np.zeros(TW, np.int64)
    for t in range(NT):
        win_tile[WOFF[t]:WOFF[t + 1]] = t
    first_win = set(int(WOFF[t]) for t in range(NT))
    last_win = set(int(WOFF[t + 1] - 1) for t in range(NT))


    nc = bacc.Bacc("TRN2", target_bir_lowering=False, debug=False,
                   num_devices=NCORES, num_swdge_queues=4)

    # I/O (weights pre-transposed / pre-broadcast on host)
    xT_in = nc.dram_tensor("xT", [IN, N], BF16, kind="ExternalInput")
    w1b_in = nc.dram_tensor("w1b", [IN, CH1], BF16, kind="ExternalInput")
    w2b_in = nc.dram_tensor("w2b", [128, 4, OUT], BF16, kind="ExternalInput")
    b1bc_in = nc.dram_tensor("b1bc", [128, CH1], F32, kind="ExternalInput")
    b2bc_in = nc.dram_tensor("b2bc", [128, OUT], F32, kind="ExternalInput")
    att2sb_in = nc.dram_tensor("att2sb", [128, OUT], F32, kind="ExternalInput")
    att2db_in = nc.dram_tensor("att2db", [128, OUT], F32, kind="ExternalInput")
    ident_in = nc.dram_tensor("ident", [128, 128], BF16, kind="ExternalInput")
    srcidx_in = nc.dram_tensor("src_idx", [128, TW * 8], I16, kind="ExternalInput")
    src2idx_in = nc.dram_tensor("src2_idx", [128, TW * 8], I16, kind="ExternalInput")
    adrows_in = nc.dram_tensor("adrows_idx", [128, NT * 8], I16, kind="ExternalInput")
    asd_in = nc.dram_tensor("asd", [N, 64], F32, kind="ExternalInput")
    S_in = nc.dram_tensor("S", [128, TW * 128], BF16, kind="ExternalInput")
    ST_in = nc.dram_tensor("ST", [128, TW * 128], BF16, kind="ExternalInput")
    y_out = nc.dram_tensor("y", [NPC, OUT], F32, kind="ExternalOutput")

    # internal DRAM
    htab = nc.dram_tensor("htab", [NROWS, HROW], BF16)
    t2piece = nc.dram_tensor("t2piece", [NPC, 128], BF16)
    t2full = nc.dram_tensor("t2full", [32 * NQ + 4, 128], BF16, addr_space="Shared")

    with tile.TileContext(nc, num_cores=NCORES) as tc:
        nc.gpsimd.load_library(library_config.mlp)
        with (
            tc.tile_pool(name="const", bufs=1) as constp,
            tc.tile_pool(name="work", bufs=2) as workp,
            tc.tile_pool(name="big", bufs=1) as bigp,
        ):
            # ---------------- phase 0: constants / setup ----------------
            w1b = constp.tile([128, CH1], BF16, tag="w1b")
            nc.sync.dma_start(w1b[:], w1b_in[:])
            w2b = constp.tile([128, 4, OUT], BF16, tag="w2b")
            nc.sync.dma_start(w2b[:], w2b_in[:])
            identb = constp.tile([128, 128], BF16, tag="identb")
            nc.sync.dma_start(identb[:], ident_in[:])
            b1bc = constp.tile([128, CH1], F32, tag="b1bc")
            nc.sync.dma_start(b1bc[:], b1bc_in[:])
            b2bc = constp.tile([128, OUT], F32, tag="b2bc")
            nc.sync.dma_start(b2bc[:], b2bc_in[:])
            att2sb = constp.tile([128, OUT], F32, tag="att2sb")
            nc.sync.dma_start(att2sb[:], att2sb_in[:])
            att2db = constp.tile([128, OUT], F32, tag="att2db")
            nc.sync.dma_start(att2db[:], att2db_in[:])

            # pad rows: h=0, a_s=BIG
            zrow = workp.tile([4, HROW], BF16, tag="zrow")
            nc.vector.memset(zrow[:], 0.0)
            nc.vector.memset(zrow[:, 512:528].bitcast(F32), BIG)
            zw = nc.sync.dma_start(htab[PAD:PAD + 4, :], zrow[:])
            prow = workp.tile([4, 128], BF16, tag="prow")
            nc.vector.memset(prow[:], 0.0)
            nc.vector.memset(prow[:, 64:72].bitcast(F32), BIG)
            nc.sync.dma_start(t2full[PAD2:PAD2 + 4, :], prow[:])

            srcidx = bigp.tile([128, TW * 8], I16, tag="srcidx")
            nc.sync.dma_start(srcidx[:], srcidx_in[:])
            src2idx = bigp.tile([128, TW * 8], I16, tag="src2idx")
            nc.sync.dma_start(src2idx[:], src2idx_in[:])
            adrows = bigp.tile([128, NT * 8], I16, tag="adrows")
            nc.sync.dma_start(adrows[:], adrows_in[:])
            a2all = bigp.tile([TILE_D, NT, 2], F32, tag="a2all")
            adall = bigp.tile([128, NT, 64], F32, tag="adall")

            # ------------- phase 1: full h-table + host a_s columns -------------
            asw = nc.sync.dma_start(htab[0:N, 512:528].bitcast(F32),
                                    asd_in[:, 0:8])
            NXT = (N + 127) // 128
            h_writes = [zw.ins, asw.ins]
            with (
                tc.tile_pool(name="pro", bufs=4) as prop,
                tc.tile_pool(name="props", bufs=4, space="PSUM") as propp,
            ):
                for i0 in range(0, NXT, 4):
                    kk = min(4, NXT - i0)
                    r0 = i0 * 128
                    nrg = min(4 * 128, N - r0)
                    xq = prop.tile([128, 512], BF16, tag="xq")
                    nc.sync.dma_start(xq[:, 0:nrg], xT_in[:, r0:r0 + nrg])
                    hsb = prop.tile([128, 4, 528], BF16, tag="hsb")
                    asb = prop.tile([128, 4, 16], F32, tag="asb")
                    for j in range(kk):
                        nr = min(128, N - (i0 + j) * 128)
                        ps_h = propp.tile([128, CH1], F32, tag="ps_h")
                        nc.tensor.matmul(ps_h[:nr, :], xq[:, j * 128:j * 128 + nr],
                                         w1b[:], start=True, stop=True)
                        nc.scalar.copy(hsb[:nr, j, 0:288], ps_h[:nr, 0:288])
                        nc.vector.tensor_copy(hsb[:nr, j, 288:512], ps_h[:nr, 288:512])
                    if nrg == kk * 128:
                        iw = nc.sync.dma_start(
                            htab[r0:r0 + nrg, 0:528].rearrange("(k p) c -> p k c", p=128),
                            hsb[:, :kk, :])
                    else:
                        iw = nc.sync.dma_start(htab[r0:r0 + nrg, 0:528], hsb[:nrg, 0, :])
                    h_writes.append(iw.ins)
                    if nrg == kk * 128:
                        ia = nc.sync.dma_start(
                            adtab[r0:r0 + nrg, 0:16].rearrange("(k p) c -> p k c", p=128),
                            asb[:, :kk, :])
                    else:
                        ia = nc.sync.dma_start(adtab[r0:r0 + nrg, 0:16], asb[:nrg, 0, :])
                    ad_writes.append(ia.ins)

            h_done = nc.vector.nop()
            for w_ in h_writes:
                add_dep_helper(h_done.ins, w_, reason="h-table complete")

            # gather own-range a rows into SBUF (input data: runs during ph1)
            for (ci, cwn) in enumerate([8, 8, 4]):
                nidx = cwn * 128
                nc.gpsimd.dma_gather(
                    adall[:, ci * 8:ci * 8 + cwn, :], asd_in[:, :],
                    adrows[:, ci * 64:ci * 64 + nidx // 16], nidx, nidx, 64)

            # ------------- phase 3: layer-1 aggregation + h2 -------------
            t2w_q = [[] for _ in range(4)]
            with (
                tc.tile_pool(name="l1g", bufs=5) as l1g,
                tc.tile_pool(name="l1", bufs=4) as l1p,
                tc.tile_pool(name="l1ps", bufs=2, space="PSUM") as l1ps,
                tc.tile_pool(name="l1ps2", bufs=2, space="PSUM") as l1ps2,
                tc.tile_pool(name="l1ps3", bufs=1, space="PSUM") as l1ps3,
            ):
                for ci in range(nchunk):
                    cw = chunk_w[ci]
                    nidx = cw * 128
                    g0 = ci * WCH
                    ioff = g0 * 8
                    gh = l1g.tile([128, WCH, HROW], BF16, tag="gh")
                    gi = nc.gpsimd.dma_gather(
                        gh[:, :cw, :], htab[:, :], srcidx[:, ioff:ioff + nidx // 16],
                        nidx, nidx, HROW, queue_num=ci % 4)
                    add_dep_helper(gi.ins, h_done.ins, reason="htab RAW")
                    ssb = l1g.tile([128, WCH, 128], BF16, tag="ssb")
                    nc.sync.dma_start(ssb[:, :cw, :], S_in[:, g0 * 128:(g0 + cw) * 128])
                    stsb = l1g.tile([128, WCH, 128], BF16, tag="stsb")
                    nc.sync.dma_start(stsb[:, :cw, :], ST_in[:, g0 * 128:(g0 + cw) * 128])
                    for wi in range(cw):
                        g = g0 + wi
                        t = int(win_tile[g])
                        if g in first_win:
                            ps_o = l1ps.tile([128, CH1], F32, tag="ps_o")
                            ps_d = l1ps2.tile([128, 8], F32, tag="ps_d")
                            adb = l1p.tile([TILE_D, 8], BF16, tag="adb")
                            nc.vector.tensor_copy(adb[:], adall[:TILE_D, t, 8:16])
                        # a_d expansion: [128 e, 8] = ST_w.T @ ad[:, 8:16]
                        ps_e = l1ps2.tile([128, 8], F32, tag="ps_e")
                        nc.tensor.matmul(ps_e[:], stsb[:TILE_D, wi, :], adb[:],
                                         start=True, stop=True)
                        ew = l1p.tile([128, 8], F32, tag="ew")
                        nc.vector.tensor_add(ew[:], ps_e[:],
                                             gh[:, wi, 512:528].bitcast(F32))
                        nc.vector.scalar_tensor_tensor(
                            ew[:], ew[:], NEG, ew[:],
                            op0=mybir.AluOpType.mult, op1=mybir.AluOpType.max)
                        expw = l1p.tile([128, 8], BF16, tag="expw")
                        nc.scalar.activation(expw[:], ew[:],
                                             mybir.ActivationFunctionType.Exp)
                        msg = l1p.tile([128, CH1], BF16, tag="msg")
                        eb = expw[:].to_broadcast((128, 8, HID))
                        nc.vector.tensor_mul(
                            msg[:].rearrange("p (h c) -> p h c", h=8),
                            gh[:, wi, 0:CH1].rearrange("p (h c) -> p h c", h=8), eb)
                        st = g in first_win
                        sp = g in last_win
                        nc.tensor.matmul(ps_o[:], ssb[:, wi, :], msg[:],
                                         start=st, stop=sp)
                        nc.tensor.matmul(ps_d[:], ssb[:, wi, :], expw[:],
                                         start=st, stop=sp)
                        if sp:
                            den = l1p.tile([TILE_D, 8], F32, tag="den")
                            nc.scalar.copy(den[:], ps_d[:TILE_D, :])
                            rec = l1p.tile([TILE_D, 8], F32, tag="rec")
                            nc.vector.reciprocal(rec[:], den[:])
                            x2 = l1p.tile([TILE_D, CH1], F32, tag="x2")
                            rb = rec[:].to_broadcast((TILE_D, 8, HID))
                            nc.vector.tensor_mul(
                                x2[:].rearrange("p (h c) -> p h c", h=8),
                                ps_o[:TILE_D, :].rearrange("p (h c) -> p h c", h=8), rb)
                            nc.vector.tensor_add(x2[:], x2[:], b1bc[:TILE_D, :])
                            x2b = l1p.tile([TILE_D, CH1], BF16, tag="x2b")
                            nc.scalar.activation(x2b[:], x2[:],
                                                 mybir.ActivationFunctionType.Relu)
                            ps_h2 = l1ps3.tile([TILE_D, OUT], F32, tag="ps_h2")
                            for k in range(4):
                                ps_x2t = l1ps3.tile([128, TILE_D], BF16, tag="ps_x2t")
                                nc.tensor.transpose(
                                    ps_x2t[:], x2b[:, k * 128:(k + 1) * 128],
                                    identb[:TILE_D, :TILE_D])
                                x2t = l1p.tile([128, TILE_D], BF16, tag="x2t")
                                nc.scalar.copy(x2t[:], ps_x2t[:])
                                nc.tensor.matmul(ps_h2[:], x2t[:], w2b[:, k, :],
                                                 start=(k == 0), stop=(k == 3))
                            h2 = l1p.tile([TILE_D, OUT], F32, tag="h2")
                            nc.vector.tensor_copy(h2[:], ps_h2[:])
                            tmp = l1p.tile([TILE_D, OUT], F32, tag="tmp")
                            nc.vector.tensor_mul(tmp[:], h2[:], att2sb[:TILE_D, :])
                            nc.vector.tensor_reduce(
                                a2all[:, t, 0:1], tmp[:], op=mybir.AluOpType.add,
                                axis=mybir.AxisListType.X)
                            nc.vector.tensor_mul(tmp[:], h2[:], att2db[:TILE_D, :])
                            nc.vector.tensor_reduce(
                                a2all[:, t, 1:2], tmp[:], op=mybir.AluOpType.add,
                                axis=mybir.AxisListType.X)
                            pc = l1p.tile([TILE_D, 128], BF16, tag="pc")
                            nc.scalar.copy(pc[:, 0:OUT], h2[:])
                            nc.vector.tensor_copy(
                                pc[:, OUT:OUT + 2].bitcast(F32), a2all[:, t, 0:1])
                            tw_ = nc.sync.dma_start(
                                t2piece[t * TILE_D:(t + 1) * TILE_D, :], pc[:])
                            t2w_q[t // 5].append(tw_.ins)

            # --------- phase 4: AllGather table2 (four quarters) ---------
            ccs = []
            for q in range(len(SEGT) - 1):
                cc = nc.gpsimd.collective_compute(
                    "AllGather", mybir.AluOpType.bypass,
                    replica_groups=[list(range(NCORES))],
                    ins=[t2piece[q * NQ:(q + 1) * NQ, :]],
                    outs=[t2full[q * 8 * NQ:(q + 1) * 8 * NQ, :]],
                )
                for w_ in t2w_q[q]:
                    add_dep_helper(cc.ins, w_, reason=f"quarter {q} ready")
                ccs.append(cc)

            # ------------- phase 6: layer-2 aggregation -------------
            with (
                tc.tile_pool(name="l2g", bufs=5) as l2g,
                tc.tile_pool(name="l2", bufs=4) as l2p,
                tc.tile_pool(name="l2ps", bufs=2, space="PSUM") as l2ps,
                tc.tile_pool(name="l2ps2", bufs=2, space="PSUM") as l2ps2,
            ):
                for ci in range(nchunk):
                    cw = chunk_w[ci]
                    nidx = cw * 128
                    g0 = ci * WCH
                    ioff = g0 * 8
                    g2 = l2g.tile([128, WCH, 128], BF16, tag="g2")
                    gi2 = nc.gpsimd.dma_gather(
                        g2[:, :cw, :], t2full[:, :], src2idx[:, ioff:ioff + nidx // 16],
                        nidx, nidx, 128, queue_num=ci % 4)
                    for cc in ccs:
                        add_dep_helper(gi2.ins, cc.ins, reason="t2full RAW")
                    ssb2 = l2g.tile([128, WCH, 128], BF16, tag="ssb2")
                    nc.sync.dma_start(ssb2[:, :cw, :], S_in[:, g0 * 128:(g0 + cw) * 128])
                    stsb2 = l2g.tile([128, WCH, 128], BF16, tag="stsb2")
                    nc.sync.dma_start(stsb2[:, :cw, :], ST_in[:, g0 * 128:(g0 + cw) * 128])
                    for wi in range(cw):
                        g = g0 + wi
                        t = int(win_tile[g])
                        if g in first_win:
                            ps_o2 = l2ps.tile([128, OUT], F32, tag="ps_o2")
                            ps_d2 = l2ps2.tile([128, 1], F32, tag="ps_d2")
                            a2b = l2p.tile([TILE_D, 1], BF16, tag="a2b")
                            nc.vector.tensor_copy(a2b[:], a2all[:, t, 1:2])
                        ps_e2 = l2ps2.tile([128, 1], F32, tag="ps_e2")
                        nc.tensor.matmul(ps_e2[:], stsb2[:TILE_D, wi, :], a2b[:],
                                         start=True, stop=True)
                        e2 = l2p.tile([128, 1], F32, tag="e2")
                        nc.vector.tensor_add(e2[:], ps_e2[:],
                                             g2[:, wi, OUT:OUT + 2].bitcast(F32))
                        nc.vector.scalar_tensor_tensor(
                            e2[:], e2[:], NEG, e2[:],
                            op0=mybir.AluOpType.mult, op1=mybir.AluOpType.max)
                        x2e = l2p.tile([128, 1], BF16, tag="x2e")
                        nc.scalar.activation(x2e[:], e2[:],
                                             mybir.ActivationFunctionType.Exp)
                        msg2 = l2p.tile([128, OUT], BF16, tag="msg2")
                        e2b = x2e[:].to_broadcast((128, 1, OUT))
                        nc.vector.tensor_mul(
                            msg2[:].rearrange("p (h c) -> p h c", h=1),
                            g2[:, wi, 0:OUT].rearrange("p (h c) -> p h c", h=1), e2b)
                        st = g in first_win
                        sp = g in last_win
                        nc.tensor.matmul(ps_o2[:], ssb2[:, wi, :], msg2[:],
                                         start=st, stop=sp)
                        nc.tensor.matmul(ps_d2[:], ssb2[:, wi, :], x2e[:],
                                         start=st, stop=sp)
                        if sp:
                            den2 = l2p.tile([TILE_D, 1], F32, tag="den2")
                            nc.scalar.copy(den2[:], ps_d2[:TILE_D, :])
                            rec2 = l2p.tile([TILE_D, 1], F32, tag="rec2")
                            nc.vector.reciprocal(rec2[:], den2[:])
                            o2 = l2p.tile([TILE_D, OUT], F32, tag="o2")
                            r2b = rec2[:].to_broadcast((TILE_D, 1, OUT))
                            nc.vector.tensor_mul(
                                o2[:].rearrange("p (h c) -> p h c", h=1),
                                ps_o2[:TILE_D, :].rearrange("p (h c) -> p h c", h=1), r2b)
                            nc.vector.tensor_add(o2[:], o2[:], b2bc[:TILE_D, :])
                            nc.sync.dma_start(
                                y_out[t * TILE_D:(t + 1) * TILE_D, :], o2[:])

    nc.compile()
    return nc


# --------------------------------------------------------------------- driver
_CACHE = {}


def kernel(x, edge_index, W1, att_src1, att_dst1, b1, W2, att_src2, att_dst2, b2):
    x = np.asarray(x); edge_index = np.asarray(edge_index)
    W1 = np.asarray(W1, np.float32); W2 = np.asarray(W2, np.float32)
    att_src1 = np.asarray(att_src1, np.float32)
    att_dst1 = np.asarray(att_dst1, np.float32)
    att_src2 = np.asarray(att_src2, np.float32)
    att_dst2 = np.asarray(att_dst2, np.float32)
    b1 = np.asarray(b1, np.float32); b2 = np.asarray(b2, np.float32)

    sched, cores = preprocess(edge_index)
    if "prog" not in _CACHE:
        _CACHE["prog"] = build_program(sched)
    nc = _CACHE["prog"]

    att1 = np.concatenate([att_src1, att_dst1], axis=0)     # [16, 64]
    watt = np.zeros((IN, 16), np.float32)
    for j in range(16):
        h = j % 8
        watt[:, j] = W1[:, h * HID:(h + 1) * HID] @ att1[j]
    # attention dots computed on host (weights x input only): [N, 16] -> pad 64
    asd = np.zeros((N, 64), np.float32)
    asd[:, 0:16] = x.astype(np.float32) @ watt

    shared = dict(
        xT=np.ascontiguousarray(x.T).astype(ml_dtypes.bfloat16),
        w1b=W1.astype(ml_dtypes.bfloat16),
        asd=asd,
        w2b=np.ascontiguousarray(
            W2.reshape(4, 128, OUT).transpose(1, 0, 2)).astype(ml_dtypes.bfloat16),
        b1bc=np.broadcast_to(b1.reshape(1, CH1), (128, CH1)).copy(),
        b2bc=np.broadcast_to(b2.reshape(1, OUT), (128, OUT)).copy(),
        att2sb=np.broadcast_to(att_src2.reshape(1, OUT), (128, OUT)).copy(),
        att2db=np.broadcast_to(att_dst2.reshape(1, OUT), (128, OUT)).copy(),
        ident=np.eye(128, dtype=ml_dtypes.bfloat16),
    )
    in_maps = []
    for c in range(NCORES):
        m = dict(shared)
        m["src_idx"] = cores[c]["src_idx"]
        m["src2_idx"] = cores[c]["src2_idx"]
        m["adrows_idx"] = cores[c]["adrows_idx"]
        m["S"] = cores[c]["S"]
        m["ST"] = cores[c]["ST"]
        in_maps.append(m)

    trace = bool(int(os.environ.get("KTRACE", "0")))
    res = run_bass_kernel_spmd(nc, in_maps, core_ids=list(range(NCORES)),
                               trace=trace)
    kernel.last_result = res
    out = np.concatenate([res.results[c]["y"] for c in range(NCORES)], axis=0)
    return out
